# revision 1
# baseline (speedup 1.0000x reference)
"""Trainium2 Bass kernel for a hybrid attention+SwiGLU transformer layer.

Strategy: pure data parallelism over B*S = 4096 tokens -> 8 shards of 512.
Each core recomputes K/V over a 1024-token halo (sliding-window attention),
so no collectives are needed. Activations are kept feature-major ("transposed",
[feature, token]) on chip so every projection matmul uses the natural weight
layout as the stationary operand and tokens as the moving free dim (N=512).
Attention scores are computed transposed (scoresT[k, q]) which makes the
whole attention block transpose-free; softmax sums over the partition axis
via ones-matmuls on the PE.

Matmuls run in bf16 with fp32 PSUM accumulation; norms/softmax/residuals in
fp32. Weights are pre-cast/pre-tiled on host; rotary tables and window masks
are host-precomputed per core.
"""
import sys, os, math

sys.path.insert(0, '/opt/trn_rl_repo')

import numpy as np
import ml_dtypes

import concourse.bass as bass
import concourse.mybir as mybir
import concourse.tile as tile
from concourse import bacc
from concourse.masks import make_identity
from concourse.bass_utils import run_bass_kernel_spmd

AF = mybir.ActivationFunctionType
DT = mybir.dt
ALU = mybir.AluOpType
BF16 = ml_dtypes.bfloat16

N_CORES = 8
EPS = 1e-6
ROPE_BASE = 10000.0
RD = 64           # rotary dim
WINDOW = 1024
EXP_BIAS = -5.0

FULL = dict(D=2048, H=16, KVH=4, FFN=8192, B=2, S=2048, OWN=512, CTX=1536)
MINI = dict(D=512, H=4, KVH=1, FFN=1024, B=2, S=2048, OWN=512, CTX=1536)

# quadrant-local 16-row half swap for stream_shuffle (rope pair exchange)
SHUF_MASK = [(i + 16) % 32 for i in range(32)]
# per-head rotary feature permutation: [e0..e15 | o0..o15 | e16..e31 | o16..o31 | 64:]
ROPE_PERM = ([2 * i for i in range(16)] + [2 * i + 1 for i in range(16)]
             + [32 + 2 * i for i in range(16)] + [33 + 2 * i for i in range(16)]
             + list(range(64, 128)))


def build_program(cfg, timing_iters=None, phases=None):
    if phases is None:
        phases = {'p1', 'q', 'kv', 'attn', 'o', 'n2', 'ffn', 'fin'}
    D, H, KVH, FFN = cfg['D'], cfg['H'], cfg['KVH'], cfg['FFN']
    OWN, CTX = cfg['OWN'], cfg['CTX']
    HD = 128
    ND = D // 128            # feature blocks of the model dim
    NF = FFN // 128          # feature blocks of the ffn dim
    NO = OWN // 128          # own token tiles (4)
    NT = CTX // 128          # context token tiles (12)
    NCH = CTX // 512         # context chunks of 512 (3)
    FG = 16 if NF >= 16 else NF
    NFG = NF // FG
    f32, bf16 = DT.float32, DT.bfloat16

    nc = bacc.Bacc("TRN2", target_bir_lowering=False, debug=False,
                   num_devices=N_CORES)
    EXT = "Internal" if timing_iters else "ExternalInput"

    # ---------------- DRAM I/O ----------------
    if timing_iters:
        tick_d = nc.dram_tensor("tick", [1, 4], DT.float32,
                                 kind="ExternalInput")
    x_ctx = nc.dram_tensor("x_ctx", [CTX, D], f32, kind=EXT)
    wq_d = nc.dram_tensor("wq", [H, 128, ND * 128], bf16, kind=EXT)
    wk_d = nc.dram_tensor("wk", [KVH, 128, ND * 128], bf16, kind=EXT)
    wv_d = nc.dram_tensor("wv", [128, ND * KVH * HD], bf16, kind=EXT)
    wo_d = nc.dram_tensor("wo", [H * HD, D], bf16, kind=EXT)
    wg_d = nc.dram_tensor("wg", [NF, 128, ND * 128], bf16, kind=EXT)
    wu_d = nc.dram_tensor("wu", [NF, 128, ND * 128], bf16, kind=EXT)
    wd_d = nc.dram_tensor("wd", [ND, 128, NF * 128], bf16, kind=EXT)
    cosq_d = nc.dram_tensor("cosq", [64, OWN], bf16, kind=EXT)
    sinq_d = nc.dram_tensor("sinq", [64, OWN], bf16, kind=EXT)
    cosk_d = nc.dram_tensor("cosk", [64, CTX], bf16, kind=EXT)
    sink_d = nc.dram_tensor("sink", [64, CTX], bf16, kind=EXT)
    mask_d = nc.dram_tensor("mask", [128, NT * OWN], bf16, kind=EXT)
    y_d = nc.dram_tensor("y", [OWN, D], f32, kind="ExternalOutput")

    rsd = 1.0 / math.sqrt(HD)

    from contextlib import ExitStack
    with tile.TileContext(nc) as tc:
        with ExitStack() as ctx:
            pool = lambda *a, **kw: ctx.enter_context(tc.tile_pool(*a, **kw))
            constp = pool(name="const", bufs=1)
            bigA = pool(name="bigA", bufs=1)      # own_hT -> t_fg
            accp = pool(name="acc", bufs=1)       # ffn down accumulator
            bigB = pool(name="bigB", bufs=1)      # attnT -> gT
            qTp = pool(name="qT", bufs=1)
            kTp = pool(name="kT", bufs=1)
            vPp = pool(name="vP", bufs=1)
            maskp = pool(name="maskp", bufs=1)
            wvresp = pool(name="wvres", bufs=1)
            wpanp = pool(name="wpan", bufs=4)     # streamed weight panels
            xf32p = pool(name="xf32", bufs=2)     # fp32 token tiles
            hbfp = pool(name="hbf", bufs=2)       # bf16 token tiles + sq scratch
            ropep = pool(name="rope", bufs=2)
            costp = pool(name="cost", bufs=1)     # cos/sin tables
            ppp = pool(name="pp", bufs=2)         # small bf16 [128,OWN] tiles
            osbp = pool(name="osb", bufs=2)
            stgp = pool(name="stg", bufs=2)       # [128,128] staging
            smlp = pool(name="sml", bufs=2)
            recpp = pool(name="recp", bufs=1)
            psp = pool(name="ps", bufs=8, space="PSUM")
            dramp = pool(name="dram", bufs=1, space="DRAM")
            identity_bf = constp.tile([128, 128], bf16, tag="idb")
            make_identity(nc, identity_bf[:])
            identity_f32 = constp.tile([128, 128], f32, tag="idf")
            make_identity(nc, identity_f32[:])
            ones_col = constp.tile([128, 1], bf16, tag="ones_col")
            nc.gpsimd.memset(ones_col[:], 1.0)
            ones_row = constp.tile([1, 128], bf16, tag="ones_row")
            nc.gpsimd.memset(ones_row[:], 1.0)
            eps_b = constp.tile([128, 1], f32, tag="eps_b")
            nc.gpsimd.memset(eps_b[:], EPS)
            expb = constp.tile([128, 1], f32, tag="expb")
            nc.gpsimd.memset(expb[:], EXP_BIAS)

            hT_dram = dramp.tile([128, ND * CTX], bf16, tag="hT")
            x2_dram = dramp.tile([OWN, D], f32, tag="x2")

            # rope tables
            cosq = costp.tile([64, OWN], bf16, tag="cq")
            sinq = costp.tile([64, OWN], bf16, tag="sq")
            cosk = costp.tile([64, CTX], bf16, tag="ck")
            sink = costp.tile([64, CTX], bf16, tag="sk")
            nc.sync.dma_start(cosq[:], cosq_d[:])
            nc.sync.dma_start(sinq[:], sinq_d[:])
            nc.sync.dma_start(cosk[:], cosk_d[:])
            nc.sync.dma_start(sink[:], sink_d[:])

            masks = maskp.tile([128, NT * OWN], bf16, tag="mask")
            nc.sync.dma_start(masks[:], mask_d[:])

            if timing_iters:
                # Internal "inputs" are uninitialized; fill them with benign
                # constants so timing is not distorted by denormals/NaNs.
                cb = hbfp.tile([128, 2048], bf16, tag="hbf")
                nc.gpsimd.memset(cb[:], 0.01)
                cf = xf32p.tile([128, 2048], f32, tag="xf32")
                nc.gpsimd.memset(cf[:], 0.01)
                for r in range(NT):
                    nc.sync.dma_start(x_ctx[r * 128:(r + 1) * 128, :],
                                      cf[:, :D])
                for hb in range(H):
                    nc.sync.dma_start(wq_d[hb], cb[:, :ND * 128])
                for kb in range(KVH):
                    nc.sync.dma_start(wk_d[kb], cb[:, :ND * 128])
                def fill_cols(dst, width, rows=128):
                    for c0 in range(0, width, 2048):
                        w = min(2048, width - c0)
                        nc.sync.dma_start(dst[:, c0:c0 + w], cb[:rows, :w])
                fill_cols(wv_d[:], ND * KVH * HD)
                for r in range(D // 128):
                    nc.sync.dma_start(wo_d[r * 128:(r + 1) * 128, :],
                                      cb[:, :D])
                for fb in range(NF):
                    nc.sync.dma_start(wg_d[fb], cb[:, :ND * 128])
                    nc.sync.dma_start(wu_d[fb], cb[:, :ND * 128])
                for ob in range(ND):
                    fill_cols(wd_d[ob], NF * 128)
                nc.sync.dma_start(cosq_d[:], cb[:64, :OWN])
                nc.sync.dma_start(sinq_d[:], cb[:64, :OWN])
                nc.sync.dma_start(cosk_d[:], cb[:64, :CTX])
                nc.sync.dma_start(sink_d[:], cb[:64, :CTX])
                fill_cols(mask_d[:], NT * OWN)
                nc.sync.dma_start(masks[:], mask_d[:])

            from contextlib import nullcontext
            loop_ctx = (tc.For_i(0, timing_iters, 1)
                        if timing_iters else nullcontext())
            with loop_ctx:
                VW = KVH * HD
                if phases & {'p1', 'q', 'kv'}:
                    own_hT = bigA.tile([128, ND * OWN], bf16, tag="bigA")
                if phases & {'q', 'attn'}:
                    qT = qTp.tile([128, H * OWN], bf16, tag="qT")
                if phases & {'kv', 'attn'}:
                    kT = kTp.tile([128, KVH * CTX], bf16, tag="kT")
                    vP = vPp.tile([128, NT * KVH * HD], bf16, tag="vP")
                if phases & {'attn', 'o'}:
                    attnT = bigB.tile([128, H * OWN], bf16, tag="bigB")
                if phases & {'n2', 'ffn'}:
                    gT = bigB.tile([128, ND * OWN], bf16, tag="bigB")
                if phases & {'ffn', 'fin'}:
                    acc = accp.tile([128, ND * OWN], f32, tag="acc")
                if len(phases) < 8:
                    # ablation builds: give every big tile >=1 write and read
                    scrap = smlp.tile([1, 2], f32, tag="scrap", bufs=1)
                    bigs = []
                    if phases & {'p1', 'q', 'kv'}: bigs.append(own_hT)
                    if phases & {'q', 'attn'}: bigs.append(qT)
                    if phases & {'kv', 'attn'}: bigs.extend([kT, vP])
                    if phases & {'attn', 'o'}: bigs.append(attnT)
                    if phases & {'n2', 'ffn'}: bigs.append(gT)
                    if phases & {'ffn', 'fin'}: bigs.append(acc)
                    for bt in bigs:
                        nc.gpsimd.memset(bt[0:1, 0:2], 0.0)
                        nc.vector.tensor_copy(scrap[0:1, 0:2], bt[0:1, 0:2])
                # ============ Phase 1: rmsnorm + transpose h ============
                if 'p1' in phases:
                    for i in range(NT):
                        own_i = i - (NT - NO)      # >=0 for own token tiles
                        xt = xf32p.tile([128, D], f32, tag="xf32")
                        nc.sync.dma_start(xt[:], x_ctx[i * 128:(i + 1) * 128, :])
                        sq = hbfp.tile([128, D], bf16, tag="hbf")
                        ss = smlp.tile([128, 1], f32, tag="ss")
                        nc.scalar.activation(sq[:], xt[:], AF.Square, accum_out=ss[:])
                        sr = smlp.tile([128, 1], f32, tag="sr")
                        nc.scalar.activation(sr[:], ss[:], AF.Sqrt, scale=1.0 / D, bias=eps_b[:])
                        rr = smlp.tile([128, 1], f32, tag="rr")
                        nc.vector.reciprocal(rr[:], sr[:])
                        ht = hbfp.tile([128, D], bf16, tag="hbf")
                        nc.vector.tensor_scalar_mul(ht[:], xt[:], rr[:])
                        for db in range(ND):
                            ptr = psp.tile([128, 128], bf16, tag="ps")
                            nc.tensor.transpose(ptr[:], ht[:, db * 128:(db + 1) * 128],
                                                identity_bf[:])
                            if own_i >= 0:
                                dst = own_hT[:, db * OWN + own_i * 128:
                                             db * OWN + (own_i + 1) * 128]
                                nc.scalar.copy(dst, ptr[:])
                                nc.sync.dma_start(
                                    hT_dram[:, db * CTX + i * 128: db * CTX + (i + 1) * 128],
                                    dst)
                            else:
                                stg = stgp.tile([128, 128], bf16, tag="hstg")
                                nc.scalar.copy(stg[:], ptr[:])
                                nc.sync.dma_start(
                                    hT_dram[:, db * CTX + i * 128: db * CTX + (i + 1) * 128],
                                    stg[:])

                # ============ Phase 2a: Q projection + rope ============
                if 'q' in phases:
                    for hb in range(H):
                        pan = wpanp.tile([128, ND * 128], bf16, tag="wpan")
                        nc.sync.dma_start(pan[:], wq_d[hb])
                        pq = psp.tile([128, OWN], f32, tag="ps")
                        for db in range(ND):
                            nc.tensor.matmul(pq[:], pan[:, db * 128:(db + 1) * 128],
                                             own_hT[:, db * OWN:(db + 1) * OWN],
                                             start=(db == 0), stop=(db == ND - 1))
                        qsl = qT[:, hb * OWN:(hb + 1) * OWN]
                        # rope rows 0:64, passthrough rows 64:128
                        qstage = ropep.tile([64, OWN], bf16, tag="rst")
                        nc.scalar.copy(qstage[:], pq[0:64, :])
                        shuf = ropep.tile([64, OWN], bf16, tag="rsh")
                        nc.vector.stream_shuffle(shuf[:], qstage[:], SHUF_MASK)
                        t1 = ropep.tile([64, OWN], bf16, tag="rt1", bufs=1)
                        nc.vector.tensor_mul(t1[:], qstage[:], cosq[:])
                        t2 = ropep.tile([64, OWN], bf16, tag="rt2", bufs=1)
                        nc.vector.tensor_mul(t2[:], shuf[:], sinq[:])
                        nc.vector.tensor_add(qsl[0:64, :], t1[:], t2[:])
                        nc.scalar.copy(qsl[64:128, :], pq[64:128, :])

                # ============ Phase 2b: K + V (streamed hT context) ============
                if 'kv' in phases:
                    wv_sb = wvresp.tile([128, ND * KVH * HD], bf16, tag="wv")
                    nc.sync.dma_start(wv_sb[:], wv_d[:])
                    kpan = []
                    for kb in range(KVH):
                        kp = wpanp.tile([128, ND * 128], bf16, tag="wpan")
                        nc.sync.dma_start(kp[:], wk_d[kb])
                        kpan.append(kp)
                    for ch in range(NCH):
                        pk = [psp.tile([128, 512], f32, tag="ps", name=f"pk{ch}_{kb}")
                              for kb in range(KVH)]
                        pv = [psp.tile([128, VW], f32, tag="ps", name=f"pv{ch}_{mi}")
                              for mi in range(4)]
                        for db in range(ND):
                            hstr = ropep.tile([128, 512], bf16, tag="hstr", bufs=3)
                            nc.sync.dma_start(
                                hstr[:], hT_dram[:, db * CTX + ch * 512:
                                                 db * CTX + (ch + 1) * 512])
                            for kb in range(KVH):
                                nc.tensor.matmul(pk[kb][:],
                                                 kpan[kb][:, db * 128:(db + 1) * 128],
                                                 hstr[:],
                                                 start=(db == 0), stop=(db == ND - 1))
                            for mi in range(4):
                                nc.tensor.matmul(pv[mi][:],
                                                 hstr[:, mi * 128:(mi + 1) * 128],
                                                 wv_sb[:, db * VW:(db + 1) * VW],
                                                 start=(db == 0), stop=(db == ND - 1))
                        for mi in range(4):
                            t_idx = ch * 4 + mi
                            nc.scalar.copy(vP[:, t_idx * VW:(t_idx + 1) * VW], pv[mi][:])
                        for kb in range(KVH):
                            ksl = kT[:, kb * CTX + ch * 512: kb * CTX + (ch + 1) * 512]
                            kstage = ropep.tile([64, 512], bf16, tag="rst")
                            nc.scalar.copy(kstage[:], pk[kb][0:64, :])
                            shuf = ropep.tile([64, 512], bf16, tag="rsh")
                            nc.vector.stream_shuffle(shuf[:], kstage[:], SHUF_MASK)
                            t1 = ropep.tile([64, 512], bf16, tag="rt1", bufs=1)
                            nc.vector.tensor_mul(t1[:], kstage[:],
                                                 cosk[:, ch * 512:(ch + 1) * 512])
                            t2 = ropep.tile([64, 512], bf16, tag="rt2", bufs=1)
                            nc.vector.tensor_mul(t2[:], shuf[:],
                                                 sink[:, ch * 512:(ch + 1) * 512])
                            nc.vector.tensor_add(ksl[0:64, :], t1[:], t2[:])
                            nc.scalar.copy(ksl[64:128, :], pk[kb][64:128, :])

                # ============ Phase 3: attention ============
                if 'attn' in phases:
                    REP = H // KVH
                    for hb in range(H):
                        kb = hb // REP
                        ap = psp.tile([128, OWN], f32, tag="ps")
                        ssum = psp.tile([1, OWN], f32, tag="ps", name=f"ssum{hb}")
                        for t in range(NT):
                            sp = psp.tile([128, OWN], f32, tag="ps")
                            nc.tensor.matmul(sp[:],
                                             kT[:, kb * CTX + t * 128: kb * CTX + (t + 1) * 128],
                                             qT[:, hb * OWN:(hb + 1) * OWN],
                                             start=True, stop=True)
                            pt = ppp.tile([128, OWN], bf16, tag="pt")
                            nc.scalar.activation(pt[:], sp[:], AF.Exp,
                                                 scale=rsd, bias=expb[:])
                            pm = ppp.tile([128, OWN], bf16, tag="pm")
                            nc.vector.tensor_mul(pm[:], pt[:],
                                                 masks[:, t * OWN:(t + 1) * OWN])
                            nc.tensor.matmul(ap[:],
                                             vP[:, t * VW + kb * HD: t * VW + (kb + 1) * HD],
                                             pm[:], start=(t == 0), stop=(t == NT - 1))
                            nc.tensor.matmul(ssum[:], ones_col[:], pm[:],
                                             start=(t == 0), stop=(t == NT - 1))
                        rec = recpp.tile([1, OWN], f32, tag="rec", bufs=1)
                        nc.vector.reciprocal(rec[:], ssum[:])
                        recb = recpp.tile([1, OWN], bf16, tag="recb", bufs=1)
                        nc.scalar.copy(recb[:], rec[:])
                        pb = psp.tile([128, OWN], f32, tag="ps")
                        nc.tensor.matmul(pb[:], ones_row[:], recb[:],
                                         start=True, stop=True)
                        asb = osbp.tile([128, OWN], bf16, tag="osb")
                        nc.scalar.copy(asb[:], ap[:])
                        nc.vector.tensor_mul(attnT[:, hb * OWN:(hb + 1) * OWN],
                                             asb[:], pb[:])

                # ============ Phase 4: O projection + residual -> x2_dram ==========
                if 'o' in phases:
                    NDC = D // 512
                    for mt in range(NO):
                        pos = [psp.tile([128, 512], f32, tag="ps",
                                        name=f"po{mt}_{dc}")
                               for dc in range(NDC)]
                        for hb in range(H):
                            pan = wpanp.tile([128, D], bf16, tag="wpan")
                            nc.sync.dma_start(pan[:],
                                              wo_d[hb * 128:(hb + 1) * 128, :])
                            a_sl = attnT[:, hb * OWN + mt * 128:
                                         hb * OWN + (mt + 1) * 128]
                            for dc in range(NDC):
                                nc.tensor.matmul(
                                    pos[dc][:], a_sl,
                                    pan[:, dc * 512:(dc + 1) * 512],
                                    start=(hb == 0), stop=(hb == H - 1))
                        row = (NT - NO + mt) * 128
                        for dc in range(NDC):
                            xs = stgp.tile([128, 512], f32, tag="xsm")
                            nc.sync.dma_start(
                                xs[:], x_ctx[row:row + 128,
                                             dc * 512:(dc + 1) * 512])
                            x2s = stgp.tile([128, 512], f32, tag="x2s")
                            nc.vector.tensor_add(x2s[:], pos[dc][:], xs[:])
                            nc.sync.dma_start(
                                x2_dram[mt * 128:(mt + 1) * 128,
                                        dc * 512:(dc + 1) * 512], x2s[:])

                # ============ Phase 5a: ffn rmsnorm + transpose g ============
                if 'n2' in phases:
                    for mt in range(NO):
                        x2t = xf32p.tile([128, D], f32, tag="xf32")
                        nc.sync.dma_start(x2t[:], x2_dram[mt * 128:(mt + 1) * 128, :])
                        sq = hbfp.tile([128, D], bf16, tag="hbf")
                        ss = smlp.tile([128, 1], f32, tag="ss")
                        nc.scalar.activation(sq[:], x2t[:], AF.Square, accum_out=ss[:])
                        sr = smlp.tile([128, 1], f32, tag="sr")
                        nc.scalar.activation(sr[:], ss[:], AF.Sqrt, scale=1.0 / D, bias=eps_b[:])
                        rr = smlp.tile([128, 1], f32, tag="rr")
                        nc.vector.reciprocal(rr[:], sr[:])
                        gt = hbfp.tile([128, D], bf16, tag="hbf")
                        nc.vector.tensor_scalar_mul(gt[:], x2t[:], rr[:])
                        for db in range(ND):
                            ptr = psp.tile([128, 128], bf16, tag="ps")
                            nc.tensor.transpose(ptr[:], gt[:, db * 128:(db + 1) * 128],
                                                identity_bf[:])
                            nc.scalar.copy(gT[:, db * OWN + mt * 128:
                                              db * OWN + (mt + 1) * 128], ptr[:])

                # ============ Phase 5b: FFN gate/up/down ============
                if 'ffn' in phases:
                    for fg in range(NFG):
                        t_fg = bigA.tile([128, FG * OWN], bf16, tag="bigA")
                        for j in range(FG):
                            fb = fg * FG + j
                            gpan = wpanp.tile([128, ND * 128], bf16, tag="wpan")
                            nc.sync.dma_start(gpan[:], wg_d[fb])
                            upan = wpanp.tile([128, ND * 128], bf16, tag="wpan")
                            nc.sync.dma_start(upan[:], wu_d[fb])
                            pg = psp.tile([128, OWN], f32, tag="ps")
                            pu = psp.tile([128, OWN], f32, tag="ps")
                            for db in range(ND):
                                nc.tensor.matmul(pg[:], gpan[:, db * 128:(db + 1) * 128],
                                                 gT[:, db * OWN:(db + 1) * OWN],
                                                 start=(db == 0), stop=(db == ND - 1))
                                nc.tensor.matmul(pu[:], upan[:, db * 128:(db + 1) * 128],
                                                 gT[:, db * OWN:(db + 1) * OWN],
                                                 start=(db == 0), stop=(db == ND - 1))
                            sg = osbp.tile([128, OWN], bf16, tag="osb")
                            nc.scalar.activation(sg[:], pg[:], AF.Sigmoid)
                            sg2 = ppp.tile([128, OWN], bf16, tag="pt")
                            nc.vector.tensor_mul(sg2[:], sg[:], pg[:])
                            nc.vector.tensor_mul(t_fg[:, j * OWN:(j + 1) * OWN],
                                                 sg2[:], pu[:])
                        for ob in range(ND):
                            dpan = wpanp.tile([128, FG * 128], bf16, tag="wpan")
                            nc.sync.dma_start(
                                dpan[:], wd_d[ob, :, fg * FG * 128:(fg + 1) * FG * 128])
                            pd = psp.tile([128, OWN], f32, tag="ps")
                            for j in range(FG):
                                nc.tensor.matmul(pd[:], dpan[:, j * 128:(j + 1) * 128],
                                                 t_fg[:, j * OWN:(j + 1) * OWN],
                                                 start=(j == 0), stop=(j == FG - 1))
                            osl = acc[:, ob * OWN:(ob + 1) * OWN]
                            if fg == 0:
                                nc.scalar.copy(osl, pd[:])
                            else:
                                nc.vector.tensor_add(osl, osl, pd[:])

                # ============ Phase 5c: transpose + final residual -> y ============
                if 'fin' in phases:
                    for mt in range(NO):
                        for og in range(D // 512):
                            ptg = psp.tile([128, 512], f32, tag="ps")
                            for k in range(4):
                                ob = og * 4 + k
                                nc.tensor.transpose(
                                    ptg[:, k * 128:(k + 1) * 128],
                                    acc[:, ob * OWN + mt * 128:
                                        ob * OWN + (mt + 1) * 128],
                                    identity_f32[:])
                            xs = stgp.tile([128, 512], f32, tag="xsm")
                            nc.sync.dma_start(
                                xs[:], x2_dram[mt * 128:(mt + 1) * 128,
                                               og * 512:(og + 1) * 512])
                            ys = stgp.tile([128, 512], f32, tag="x2s")
                            nc.vector.tensor_add(ys[:], ptg[:], xs[:])
                            nc.sync.dma_start(
                                y_d[mt * 128:(mt + 1) * 128,
                                    og * 512:(og + 1) * 512], ys[:])

    nc.compile()
    return nc


# ---------------------------------------------------------------------------
# Host-side preparation
# ---------------------------------------------------------------------------

def _rope_tables(pos, dtype=BF16):
    """Build the [64, m] A (cos) and B (+-sin) tables for the permuted layout."""
    inv_freq = 1.0 / (ROPE_BASE ** (np.arange(0, RD, 2, dtype=np.float64) / RD))
    ang = inv_freq[:, None] * pos[None, :].astype(np.float64)   # [32, m]
    cos, sin = np.cos(ang), np.sin(ang)
    rmap = np.concatenate([np.arange(16), np.arange(16),
                           np.arange(16, 32), np.arange(16, 32)])
    sign = np.ones(64); sign[0:16] = -1.0; sign[32:48] = -1.0
    A = cos[rmap]                       # [64, m]
    B = sign[:, None] * sin[rmap]
    return A.astype(dtype), B.astype(dtype)


def prep_inputs(cfg, x, position_ids, attn_norm_w, wq, wk, wv, wo, ffn_norm_w,
                w_gate, w_up, w_down):
    D, H, KVH, FFN = cfg['D'], cfg['H'], cfg['KVH'], cfg['FFN']
    B, S, OWN, CTX = cfg['B'], cfg['S'], cfg['OWN'], cfg['CTX']
    HD = 128
    ND, NF, NT = D // 128, FFN // 128, CTX // 128
    NCHUNK = S // OWN

    x = np.asarray(x, np.float32)
    anw = np.asarray(attn_norm_w, np.float32)
    fnw = np.asarray(ffn_norm_w, np.float32)
    perm = np.asarray(ROPE_PERM)

    def panelize(w, nout):
        # w: [D_in, NOUT*128] -> [NOUT, 128, ND_in*128] panel image
        # pan[ob, p, n*128+c] = w[n*128+p, ob*128+c]
        din = w.shape[0]
        ndin = din // 128
        return np.ascontiguousarray(
            w.reshape(ndin, 128, nout, 128).transpose(2, 1, 0, 3)
            .reshape(nout, 128, ndin * 128))

    wq_f = (np.asarray(wq, np.float32) * anw[:, None]).reshape(D, H, HD)
    wq_f = wq_f[:, :, perm].reshape(D, H * HD)
    wq_t = panelize(wq_f, H).astype(BF16)
    wk_f = (np.asarray(wk, np.float32) * anw[:, None]).reshape(D, KVH, HD)
    wk_f = wk_f[:, :, perm].reshape(D, KVH * HD)
    wk_t = panelize(wk_f, KVH).astype(BF16)
    VW = KVH * HD
    wv_f = np.asarray(wv, np.float32) * anw[:, None]
    wv_t = np.ascontiguousarray(
        wv_f.reshape(ND, 128, VW).transpose(1, 0, 2)
        .reshape(128, ND * VW)).astype(BF16)
    wo_t = np.ascontiguousarray(np.asarray(wo, np.float32)).astype(BF16)
    wg_t = panelize(np.asarray(w_gate, np.float32) * fnw[:, None], NF).astype(BF16)
    wu_t = panelize(np.asarray(w_up, np.float32) * fnw[:, None], NF).astype(BF16)
    wd_t = panelize(np.asarray(w_down, np.float32), ND).astype(BF16)

    pos_ids = np.asarray(position_ids)

    in_maps = []
    for s in range(N_CORES):
        b, c = divmod(s, NCHUNK)
        lo = c * OWN - (CTX - OWN)          # global start of ctx window
        x_c = np.zeros((CTX, D), np.float32)
        g0, g1 = max(0, lo), c * OWN + OWN
        x_c[g0 - lo: g1 - lo] = x[b, g0:g1]

        posq = np.asarray(pos_ids[b, c * OWN: c * OWN + OWN], np.float64)
        posk_idx = np.clip(np.arange(lo, lo + CTX), 0, S - 1)
        posk = np.asarray(pos_ids[b], np.float64)[posk_idx]
        cosq, sinq = _rope_tables(posq)
        cosk, sink = _rope_tables(posk)

        j = np.arange(CTX)[:, None]         # local key index
        qi = np.arange(OWN)[None, :]
        valid = (j >= qi + 1) & (j <= qi + WINDOW) & (j >= (g0 - lo))
        mask = np.ascontiguousarray(
            valid.astype(BF16).reshape(NT, 128, OWN).transpose(1, 0, 2)
            .reshape(128, NT * OWN))

        in_maps.append(dict(
            x_ctx=x_c, wq=wq_t, wk=wk_t, wv=wv_t, wo=wo_t,
            wg=wg_t, wu=wu_t, wd=wd_t,
            cosq=cosq, sinq=sinq, cosk=cosk, sink=sink, mask=mask))
    return in_maps


_NC_CACHE = {}


def _get_nc(cfg_key='full'):
    if cfg_key not in _NC_CACHE:
        _NC_CACHE[cfg_key] = build_program(FULL if cfg_key == 'full' else MINI)
    return _NC_CACHE[cfg_key]


def kernel(**inputs):
    cfg = FULL
    nc = _get_nc('full')
    in_maps = prep_inputs(cfg, **inputs)
    res = run_bass_kernel_spmd(nc, in_maps, list(range(N_CORES)))
    B, S, D, OWN = cfg['B'], cfg['S'], cfg['D'], cfg['OWN']
    NCHUNK = S // OWN
    out = np.empty((B, S, D), np.float32)
    for s in range(N_CORES):
        b, c = divmod(s, NCHUNK)
        out[b, c * OWN:(c + 1) * OWN] = res.results[s]["y"]
    return out

